# revision 13
# baseline (speedup 1.0000x reference)
"""2D DCT [8,32,256,256] on 8 TRN2 NeuronCores.

Math: the reference's FFT-mirror trick is exactly the linear map
    dct1d(x)[k] = (1/L) * sum_m x[m] * cos(pi*k*(m+0.5)/L)
so with A[m,k] = cos(pi*k*(m+0.5)/L)/L the 2D DCT per [256,256] slice is
    out = A^T @ X @ A = (X^T A)^T A
which is two chained TensorEngine matmuls with NO transposes:
    V  = matmul(lhsT=X, rhs=A)   # V = X^T A   (V lands [w, j] in PSUM)
    out= matmul(lhsT=V, rhs=A)   # V^T A = A^T X A  ([h', w'] in PSUM)

Sharding: fully data-parallel over the batch dim — core b takes ip[b]
(32 independent [256,256] slices). Input/output are staged as bf16 with a
host-side layout [128, 32, 2, 256] so every DMA line is contiguous per
partition; matmuls run bf16 with f32 PSUM accumulation.

Constraint in this toolchain: a lowered DMA instruction supports at most
ONE sync wait. So: all input tiles are resident (no recycle wait), each
DMA lane (8 HWDGE sems via sync, 8 SWDGE via gpsimd) is used at most
twice with the reuse carrying no data dep, and each output staging tile
has a single writer engine (one-sem wait for its DMA).
"""

import numpy as np

import concourse.bacc as bacc
import concourse.bass as bass
import concourse.mybir as mybir
import concourse.tile as tile
from concourse.bass_utils import run_bass_kernel_spmd

N_CORES = 8
C = 32                    # slices per core (channel dim; batch is sharded)
L = 256                   # DCT length
BF16 = mybir.dt.bfloat16
F32 = mybir.dt.float32
NP_BF16 = mybir.dt.np(mybir.dt.bfloat16)

# Slices per in/out DMA chunk. Small leading in-chunks start compute
# early; small trailing out-chunks shrink the drain tail. 8 chunks each
# so the 8 HWDGE lanes serve the ins and the 8 SWDGE lanes the outs.
IN_CHUNKS = [1, 1, 2, 4, 6, 6, 6, 6]
OUT_CHUNKS = [6, 6, 6, 6, 4, 2, 1, 1]


def _dct_matrix() -> np.ndarray:
    m = np.arange(L, dtype=np.float64)
    k = np.arange(L, dtype=np.float64)
    a = np.cos(np.pi * np.outer(m + 0.5, k) / L) / L
    return a.astype(np.float32).astype(NP_BF16)


def _build() -> bass.Bass:
    nc = bacc.Bacc()
    x = nc.declare_dram_parameter("x", [128, C, 2, L], BF16, isOutput=False)
    a = nc.declare_dram_parameter("dct", [L, L], BF16, isOutput=False)
    out = nc.declare_dram_parameter("out", [128, C, 2, L], BF16, isOutput=True)

    with tile.TileContext(nc) as tc:
        with (
            tc.tile_pool(name="const", bufs=1) as const_pool,
            tc.tile_pool(name="xin", bufs=1) as x_pool,
            tc.tile_pool(name="vsb", bufs=6) as v_pool,
            tc.tile_pool(name="osb", bufs=1) as o_pool,
            tc.tile_pool(name="vps", bufs=2, space="PSUM") as vps_pool,
            tc.tile_pool(name="ops", bufs=2, space="PSUM") as ops_pool,
            tc.tile_pool(name="wps", bufs=1, space="PSUM") as warm_pool,
        ):
            # HAM warm-up: ~28 dummy matmuls on garbage SBUF fill the
            # PE during the input-DMA head so real matmuls run at 2.4
            # GHz from the first slice.
            warm_sb = const_pool.tile([128, 128], BF16)
            warm_ps = warm_pool.tile([128, 128], F32)
            nc.any.memset(warm_sb[:], 0.0)
            for _ in range(28):
                nc.tensor.matmul(
                    warm_ps[:], warm_sb[:], warm_sb[:], start=True, stop=True
                )

            # A rows ki*128+p land on partition p, chunk ki. Issued on
            # the ACT HWDGE ring so it doesn't serialize with the input
            # chunks on the sync ring.
            a_sb = const_pool.tile([128, 2, L], BF16)
            nc.scalar.dma_start(a_sb[:], a.rearrange("(ki p) w -> p ki w", p=128))

            # All 32 slices stay resident (32KB/partition) — fresh tiles
            # per chunk, so in-DMAs carry no recycle wait.
            xs_tiles = {}
            c0 = 0
            for ci, n in enumerate(IN_CHUNKS):
                xt = x_pool.tile([128, n, 2, L], BF16, tag=f"x{ci}")
                nc.sync.dma_start(xt[:], x[:, c0 : c0 + n, :, :])
                for sc in range(n):
                    xs_tiles[c0 + sc] = (xt, sc)
                c0 += n

            os_tiles = {}
            c0 = 0
            for ci, n in enumerate(OUT_CHUNKS):
                ot = o_pool.tile([128, n, 2, L], BF16, tag=f"o{ci}")
                for sc in range(n):
                    os_tiles[c0 + sc] = (ot, sc, ci, c0 + n - 1)
                c0 += n

            for s in range(C):
                xt, xsc = xs_tiles[s]
                ot, osc, oci, olast = os_tiles[s]
                # One whole-bank PSUM tile and one big eviction per
                # stage; vs/os evictions split across DVE and ACT.
                vs_copy = nc.vector.tensor_copy if s % 2 else nc.scalar.copy
                os_copy = nc.scalar.copy if s % 2 else nc.vector.tensor_copy
                vs = v_pool.tile([128, 2, L], BF16, tag="vs")
                vp = vps_pool.tile([128, 2, L], F32, tag="vp")
                for mi in range(2):
                    for ki in range(2):
                        nc.tensor.matmul(
                            vp[:, mi, :],
                            xt[:, xsc, ki, mi * 128 : (mi + 1) * 128],
                            a_sb[:, ki, :],
                            start=(ki == 0),
                            stop=(ki == 1),
                        )
                vs_copy(vs[:], vp[:])
                op = ops_pool.tile([128, 2, L], F32, tag="op")
                for ji in range(2):
                    for wi in range(2):
                        nc.tensor.matmul(
                            op[:, ji, :],
                            vs[:, wi, ji * 128 : (ji + 1) * 128],
                            a_sb[:, wi, :],
                            start=(wi == 0),
                            stop=(wi == 1),
                        )
                os_copy(ot[:, osc, :, :], op[:])
                if s == olast:
                    # Whole out-chunk staged; SWDGE DMA (gpsimd) keeps
                    # the outs off the input HWDGE ring and off the
                    # copy engines (inline DMA waits would stall them).
                    lo = s + 1 - OUT_CHUNKS[oci]
                    nc.gpsimd.dma_start(out[:, lo : s + 1, :, :], ot[:])
    nc.compile()
    return nc


_NC_CACHE: bass.Bass | None = None


def _get_nc() -> bass.Bass:
    global _NC_CACHE
    if _NC_CACHE is None:
        _NC_CACHE = _build()
    return _NC_CACHE


def _make_in_maps(ip: np.ndarray) -> list[dict[str, np.ndarray]]:
    a = _dct_matrix()
    in_maps = []
    for b in range(N_CORES):
        xb = ip[b].astype(NP_BF16)                     # [C, 256, 256]
        xb = xb.reshape(C, 2, 128, L).transpose(2, 0, 1, 3)  # [128, C, 2, L]
        in_maps.append({"x": np.ascontiguousarray(xb), "dct": a})
    return in_maps


def _unpack_out(results: list[dict[str, np.ndarray]]) -> np.ndarray:
    outs = []
    for b in range(N_CORES):
        ob = np.asarray(results[b]["out"])             # [128, C, 2, L] bf16
        ob = ob.transpose(1, 2, 0, 3).reshape(C, 256, 256).astype(np.float32)
        outs.append(ob)
    return np.stack(outs, axis=0)


def run(ip: np.ndarray, trace: bool = False):
    """Run the device kernel; returns (output, BassKernelResults)."""
    ip = np.asarray(ip)
    assert ip.shape == (N_CORES, C, 256, 256), ip.shape
    res = run_bass_kernel_spmd(
        _get_nc(), _make_in_maps(ip), core_ids=list(range(N_CORES)), trace=trace
    )
    return _unpack_out(res.results), res


def kernel(ip: np.ndarray) -> np.ndarray:
    out, _ = run(ip)
    return out


# revision 18
# speedup vs baseline: 1.0968x; 1.0968x over previous
"""2D DCT [8,32,256,256] on 8 TRN2 NeuronCores — raw Bass (no Tile).

Math: the reference's FFT-mirror trick is exactly the linear map
    dct1d(x)[k] = (1/L) * sum_m x[m] * cos(pi*k*(m+0.5)/L)
so with A[m,k] = cos(pi*k*(m+0.5)/L)/L the 2D DCT per [256,256] slice is
    out = A^T @ X @ A = (X^T A)^T A
i.e. two chained TensorEngine matmuls with NO transposes:
    V  = matmul(lhsT=X, rhs=A)   # V = X^T A   ([w, j] in PSUM)
    out= matmul(lhsT=V, rhs=A)   # V^T A = A^T X A  ([h', w'] in PSUM)

Sharding: fully data-parallel over batch — core b takes ip[b] (32
independent slices). bf16 staging in a [128, 32, 2, 256] host layout
(contiguous per partition), f32 PSUM accumulation.

Raw-Bass engine plan (Tile's entry/exit barriers cost ~10us):
  SP (sync)  : HWDGE ring — const A, 8 graduated in-chunks, then the 8
               out-chunks (issue stalls on copy sems, ring stays FIFO),
               final wait for out completions.
  PE         : warm-up matmuls (HAM), then software-pipelined
               S1(0..K-1), [S1(s), S2(s-K)]..., S2 tail; one sem inc
               per stage.
  DVE / ACT  : PSUM->SBUF evictions, one whole-bank copy per stage,
               alternating engines per slice.
"""

import numpy as np

import concourse.bacc as bacc
import concourse.bass as bass
import concourse.mybir as mybir
from concourse.bass_utils import run_bass_kernel_spmd

N_CORES = 8
C = 32                    # slices per core (channel dim; batch is sharded)
L = 256                   # DCT length
BF16 = mybir.dt.bfloat16
F32 = mybir.dt.float32
NP_BF16 = mybir.dt.np(mybir.dt.bfloat16)

IN_CHUNKS = [1, 1, 2, 4, 6, 6, 6, 6]
OUT_CHUNKS = [6, 6, 6, 6, 4, 2, 1, 1]
N_WARM = 28               # HAM warm-up matmuls during the DMA head
PS_R = 3                  # vp/op PSUM ring depth (banks each)
VS_R = 6                  # vs SBUF ring depth
LOOKAHEAD = PS_R          # S2(s) issues after S1(s+LOOKAHEAD-? ) — see loop


def _dct_matrix() -> np.ndarray:
    m = np.arange(L, dtype=np.float64)
    k = np.arange(L, dtype=np.float64)
    a = np.cos(np.pi * np.outer(m + 0.5, k) / L) / L
    return a.astype(np.float32).astype(NP_BF16)


def _chunk_of(s):
    c0 = 0
    for ci, n in enumerate(IN_CHUNKS):
        if s < c0 + n:
            return ci, c0
        c0 += n
    raise AssertionError


def _pe_schedule():
    """PE stage emission order and the pe_sem value after each stage."""
    order = []
    for s in range(C):
        order.append(("S1", s))
        if s >= LOOKAHEAD:
            order.append(("S2", s - LOOKAHEAD))
    for s in range(C - LOOKAHEAD, C):
        order.append(("S2", s))
    pe_count = {st: i + 1 for i, st in enumerate(order)}
    return order, pe_count


def _copy_plan(pe_count):
    """Copy events: engine, pe_sem dep, and per-engine stream position.

    vs_copy(s) evicts stage-1 PSUM (dep: S1(s) done); os_copy(s) evicts
    stage-2 (dep: S2(s) done). Engines alternate by slice parity; each
    engine's stream is sorted by dep so no head-of-line blocking.
    """
    events = []
    for s in range(C):
        events.append(("vs", s, "dve" if s % 2 == 0 else "act", pe_count[("S1", s)]))
        events.append(("os", s, "act" if s % 2 == 0 else "dve", pe_count[("S2", s)]))
    streams = {"dve": [], "act": []}
    for kind, s, eng, dep in events:
        streams[eng].append((dep, kind, s))
    pos = {}
    for eng, evs in streams.items():
        evs.sort()
        for i, (dep, kind, s) in enumerate(evs):
            pos[(kind, s)] = (eng, i + 1, dep)
    return streams, pos


def _build() -> bass.Bass:
    nc = bacc.Bacc()
    x = nc.declare_dram_parameter("x", [128, C, 2, L], BF16, isOutput=False)
    a = nc.declare_dram_parameter("dct", [L, L], BF16, isOutput=False)
    out = nc.declare_dram_parameter("out", [128, C, 2, L], BF16, isOutput=True)

    order, pe_count = _pe_schedule()
    streams, pos = _copy_plan(pe_count)

    from contextlib import ExitStack

    ctx = ExitStack()
    with ctx:
        a_sb = ctx.enter_context(nc.sbuf_tensor([128, 2, L], BF16))
        warm_sb = ctx.enter_context(nc.sbuf_tensor([128, 128], BF16))
        xs = ctx.enter_context(nc.sbuf_tensor([128, C, 2, L], BF16))
        vs = ctx.enter_context(nc.sbuf_tensor([128, VS_R, 2, L], BF16))
        os_ = ctx.enter_context(nc.sbuf_tensor([128, C, 2, L], BF16))
        vp = ctx.enter_context(nc.psum_tensor([128, PS_R, 2, L], F32))
        op = ctx.enter_context(nc.psum_tensor([128, PS_R, 2, L], F32))
        warm_ps = ctx.enter_context(nc.psum_tensor([128, 128], F32))

        const_sem = ctx.enter_context(nc.semaphore("const_sem"))
        in_sems = [
            ctx.enter_context(nc.semaphore(f"in_sem{i}"))
            for i in range(len(IN_CHUNKS))
        ]
        pe_sem = ctx.enter_context(nc.semaphore("pe_sem"))
        dve_sem = ctx.enter_context(nc.semaphore("dve_sem"))
        act_sem = ctx.enter_context(nc.semaphore("act_sem"))
        out_sem = ctx.enter_context(nc.semaphore("out_sem"))
        warm_sem = ctx.enter_context(nc.semaphore("warm_sem"))
        sem_of = {"dve": dve_sem, "act": act_sem}

        block = ctx.enter_context(nc.Block())

        @block.sync
        def _(eng):
            # const A first (needed with in0 for the first real matmul)
            eng.dma_start(
                a_sb[:], a.rearrange("(ki p) w -> p ki w", p=128)
            ).then_inc(const_sem, 16)
            c0 = 0
            for ci, n in enumerate(IN_CHUNKS):
                eng.dma_start(
                    xs[:, c0 : c0 + n, :, :], x[:, c0 : c0 + n, :, :]
                ).then_inc(in_sems[ci], 16)
                c0 += n
            # out-chunks on the same HWDGE ring; issue blocks on the
            # staging copies, the ring itself stays busy with ins.
            c0 = 0
            for n in OUT_CHUNKS:
                for eng_name in ("dve", "act"):
                    need = max(
                        (
                            pos[("os", s)][1]
                            for s in range(c0, c0 + n)
                            if pos[("os", s)][0] == eng_name
                        ),
                        default=0,
                    )
                    if need:
                        eng.wait_ge(sem_of[eng_name], need)
                eng.dma_start(
                    out[:, c0 : c0 + n, :, :], os_[:, c0 : c0 + n, :, :]
                ).then_inc(out_sem, 16)
                c0 += n
            eng.wait_ge(out_sem, 16 * len(OUT_CHUNKS))

        @block.tensor
        def _(eng):
            eng.wait_ge(warm_sem, 1)
            for _ in range(N_WARM):
                nc.tensor.matmul(
                    warm_ps[:], warm_sb[:], warm_sb[:], start=True, stop=True
                )
            eng.wait_ge(const_sem, 16)
            for kind, s in order:
                r = s % PS_R
                if kind == "S1":
                    ci, clo = _chunk_of(s)
                    if s == clo:
                        eng.wait_ge(in_sems[ci], 16)
                    if s >= PS_R:
                        # vp ring slot reuse: vs_copy(s-PS_R) done
                        e, p, _ = pos[("vs", s - PS_R)]
                        eng.wait_ge(sem_of[e], p)
                    for mi in range(2):
                        for ki in range(2):
                            mm = nc.tensor.matmul(
                                vp[:, r, mi, :],
                                xs[:, s, ki, mi * 128 : (mi + 1) * 128],
                                a_sb[:, ki, :],
                                start=(ki == 0),
                                stop=(ki == 1),
                            )
                    mm.then_inc(pe_sem, 1)
                else:
                    e, p, _ = pos[("vs", s)]          # vs(s) staged
                    eng.wait_ge(sem_of[e], p)
                    if s >= PS_R:
                        # op ring slot reuse: os_copy(s-PS_R) done
                        e, p, _ = pos[("os", s - PS_R)]
                        eng.wait_ge(sem_of[e], p)
                    for ji in range(2):
                        for wi in range(2):
                            mm = nc.tensor.matmul(
                                op[:, r, ji, :],
                                vs[:, s % VS_R, wi, ji * 128 : (ji + 1) * 128],
                                a_sb[:, wi, :],
                                start=(wi == 0),
                                stop=(wi == 1),
                            )
                    mm.then_inc(pe_sem, 1)

        def copy_stream(eng_name):
            def body(eng):
                copy = (
                    nc.vector.tensor_copy if eng_name == "dve" else nc.scalar.copy
                )
                if eng_name == "dve":
                    nc.vector.memset(warm_sb[:], 0.0).then_inc(warm_sem, 1)
                for dep, kind, s in streams[eng_name]:
                    eng.wait_ge(pe_sem, dep)
                    if kind == "vs":
                        copy(vs[:, s % VS_R, :, :], vp[:, s % PS_R, :, :]).then_inc(
                            sem_of[eng_name], 1
                        )
                    else:
                        copy(os_[:, s, :, :], op[:, s % PS_R, :, :]).then_inc(
                            sem_of[eng_name], 1
                        )
            return body

        block.vector(copy_stream("dve"))
        block.scalar(copy_stream("act"))

    nc.compile()
    return nc


_NC_CACHE: bass.Bass | None = None


def _get_nc() -> bass.Bass:
    global _NC_CACHE
    if _NC_CACHE is None:
        _NC_CACHE = _build()
    return _NC_CACHE


def _make_in_maps(ip: np.ndarray) -> list[dict[str, np.ndarray]]:
    a = _dct_matrix()
    in_maps = []
    for b in range(N_CORES):
        xb = ip[b].astype(NP_BF16)                     # [C, 256, 256]
        xb = xb.reshape(C, 2, 128, L).transpose(2, 0, 1, 3)  # [128, C, 2, L]
        in_maps.append({"x": np.ascontiguousarray(xb), "dct": a})
    return in_maps


def _unpack_out(results: list[dict[str, np.ndarray]]) -> np.ndarray:
    outs = []
    for b in range(N_CORES):
        ob = np.asarray(results[b]["out"])             # [128, C, 2, L] bf16
        ob = ob.transpose(1, 2, 0, 3).reshape(C, 256, 256).astype(np.float32)
        outs.append(ob)
    return np.stack(outs, axis=0)


def run(ip: np.ndarray, trace: bool = False):
    """Run the device kernel; returns (output, BassKernelResults)."""
    ip = np.asarray(ip)
    assert ip.shape == (N_CORES, C, 256, 256), ip.shape
    res = run_bass_kernel_spmd(
        _get_nc(), _make_in_maps(ip), core_ids=list(range(N_CORES)), trace=trace
    )
    return _unpack_out(res.results), res


def kernel(ip: np.ndarray) -> np.ndarray:
    out, _ = run(ip)
    return out
